# revision 1
# baseline (speedup 1.0000x reference)
"""BERT self-attention (B=4, S=2048, H=768, 12 heads x d=64) on 8 Trainium2
NeuronCores.

Sharding: core c handles batch b = c//2 and head group hg = c%2 (6 heads).
No cross-core communication; the host scatters inputs and gathers the output.

v2 vs baseline (350us):
  - inputs fed as bf16 (halves input DMA and LDWEIGHTS cost; PE rate for
    fp32r with N>=256 was already 1 cycle/row so matmul time is unchanged)
  - zero bias => contraction is exactly 6 chunks of 128 (768), no pad chunk
  - consecutive matmuls share the loaded stationary operand where possible
    (the ISA caps one matmul at 512 moving elements, so multi-bank streams
    are separate instructions; the 2nd LDWEIGHTS mostly hides under the
    1st matmul's stream when its semaphore waits are already satisfied)
  - softmax exp as one 2048-elem ACTIVATE per j-tile (4 psum banks:
    2 heads x 2 i-blocks) cutting scalar-engine per-instruction overhead
  - ctx psum staged to SBUF with fast DVE copies so the psum ring frees
    quickly; softmax division done off-ring (DVE reciprocal + gpsimd mults)
    (note: reciprocal_approx_fast passes CoreSim but returns garbage on HW)
  - chunk = (head pair, i-half of 1024); per j-tile of 128 tokens:
    scores (2 row-group-concurrent compound matmuls) -> exp -> ctx of the
    previous chunk (compound over the 2 i-blocks per v load)

PSUM budget (8 banks): scores/proj ring "s" [128,4,512] x1 = 4 banks;
ctx/v/fill ring "c" [128,2,512] x2 = 4 banks.  Projection fills that need
psum are scheduled only where a ring is free: pair0+pair1-q in the startup
window, v during chunk0 (no ctx yet), pair1-k/pair2 at chunk boundaries
between a finalize and the next ctx accumulation.

Per-core layouts (SBUF [128 partitions x free]):
  xT   [128, 6, 2048] bf16   x[b].T
  wq/wk/wv [128, 6, 384] bf16 weight column-slices for this head group
  qT/kT [128, 3, 2048] bf16  per head-pair stacked d-dims (even head: p0-63,
                             odd: p64-127)
  v    [128, 16, 6, 96] bf16 token-major v; cols 64:96 of each head are ones
                             so the ctx matmul leaves 32 copies of sumexp in
                             psum rows 64:96 (free softmax denominator)
  ss psum [128, 2h, 2ic, 512] scores -> exp -> ex sbuf bf16
  pc psum [96, 2ic, 512] ctx^T rows 0:64, sumexp copies rows 64:96
"""
import os

import numpy as np

if not os.environ.get("KERNEL_TRACE"):
    os.environ.setdefault("BASS_NEVER_TRACE", "1")

import concourse.bass as bass
import concourse.mybir as mybir
import concourse.tile as tile
from concourse import bacc
from concourse.bass import ts
from concourse.bass_utils import run_bass_kernel_spmd

import ml_dtypes

F32 = mybir.dt.float32
BF16 = mybir.dt.bfloat16

HIDDEN = 768
N_HEADS = 12
HEAD_DIM = 64
B = 4
S = 2048
HPC = 6          # heads per core
NPAIR = HPC // 2
NJ = S // 128    # 16 j-tiles of 128 tokens
VW = 96          # v (64) | ones (32)

_cache = {}
last_results = None


def _build(use_mask: bool, use_bias: bool):
    KC = 7 if use_bias else 6   # contraction chunks of 128
    nc = bacc.Bacc("TRN2", target_bir_lowering=False, debug=False, num_devices=8)

    xT_d = nc.dram_tensor("xT", [KC * 128, S], BF16, kind="ExternalInput")
    wq_d = nc.dram_tensor("wq", [KC * 128, HPC * HEAD_DIM], BF16, kind="ExternalInput")
    wk_d = nc.dram_tensor("wk", [KC * 128, HPC * HEAD_DIM], BF16, kind="ExternalInput")
    wv_d = nc.dram_tensor("wv", [KC * 128, HPC * HEAD_DIM], BF16, kind="ExternalInput")
    if use_mask:
        em_d = nc.dram_tensor("em", [128, NJ], F32, kind="ExternalInput")
    out_d = nc.dram_tensor("out", [HPC, HEAD_DIM, S], F32, kind="ExternalOutput")

    with tile.TileContext(nc) as tc:
        with (
            tc.tile_pool(name="const", bufs=1) as cpool,
            tc.tile_pool(name="qk", bufs=1) as qkpool,
            tc.tile_pool(name="vp", bufs=1) as vpool,
            tc.tile_pool(name="op", bufs=1) as opool,
            tc.tile_pool(name="rp", bufs=2) as rpool,
            tc.tile_pool(name="st", bufs=2) as stpool,
            tc.tile_pool(name="xw", bufs=1) as xwpool,
            tc.tile_pool(name="ex", bufs=12) as expool,
            tc.tile_pool(name="pss", bufs=1, space="PSUM") as pss,
            tc.tile_pool(name="psc", bufs=2, space="PSUM") as psc,
        ):
            if use_mask:
                em = cpool.tile([128, NJ], F32)
                nc.sync.dma_start(em[:], em_d[:])

            qT = qkpool.tile([128, NPAIR, S], BF16)
            kT = qkpool.tile([128, NPAIR, S], BF16)
            v = vpool.tile([128, NJ, HPC, VW], BF16)
            nc.vector.memset(v[:, :, :, HEAD_DIM:VW], 1.0)

            xT = xwpool.tile([128, KC, S], BF16)
            wq = xwpool.tile([128, KC, HPC * HEAD_DIM], BF16)
            wk = xwpool.tile([128, KC, HPC * HEAD_DIM], BF16)
            wv = xwpool.tile([128, KC, HPC * HEAD_DIM], BF16)
            # weights first (small), then x chunk-by-chunk so the first
            # projection matmuls start as soon as each chunk lands
            for c in range(KC):
                nc.sync.dma_start(wq[:, c, :], wq_d[ts(c, 128), :])
                nc.sync.dma_start(wk[:, c, :], wk_d[ts(c, 128), :])
            for c in range(KC):
                nc.sync.dma_start(xT[:, c, :], xT_d[ts(c, 128), :])
            for c in range(KC):
                nc.sync.dma_start(wv[:, c, :], wv_d[ts(c, 128), :])

            def emit_qk_pss(p, which):
                # one 4-bank psum group: per c, 4 N=512 matmuls sharing one
                # loaded stationary (ldw-opt elides the repeat LDWEIGHTS)
                w_, dst = (wq, qT) if which == 0 else (wk, kT)
                acc = pss.tile([128, 4, 512], F32, tag="s", name=f"qk{p}{which}")
                for c in range(KC):
                    for n in range(4):
                        nc.tensor.matmul(
                            acc[:, n, :], w_[:, c, ts(p, 128)],
                            xT[:, c, ts(n, 512)],
                            start=(c == 0), stop=(c == KC - 1),
                        )
                nc.vector.tensor_copy(
                    dst[:, p, :], acc[:].rearrange("p a n -> p (a n)")
                )

            def emit_qk_psc(p, which, halves=(0, 1)):
                # projection as 2-bank half-groups on the "c" ring
                w_, dst = (wq, qT) if which == 0 else (wk, kT)
                for half in halves:
                    acc = psc.tile([128, 2, 512], F32, tag="c",
                                   name=f"qkh{p}{which}{half}")
                    for c in range(KC):
                        for n in range(2):
                            nc.tensor.matmul(
                                acc[:, n, :], w_[:, c, ts(p, 128)],
                                xT[:, c, ts(2 * half + n, 512)],
                                start=(c == 0), stop=(c == KC - 1),
                            )
                    nc.vector.tensor_copy(
                        dst[:, p, ts(half, 1024)],
                        acc[:].rearrange("p a n -> p (a n)"),
                    )

            def emit_qk_first():
                # pair0 token-half 0 only: chunk0 (ic2=0) scores read just
                # qT tokens 0:1024, so the first activation starts ~7us
                # earlier; the second halves follow as early chunk-0 fills
                qacc = pss.tile([128, 2, 512], F32, tag="s", name="q0acc")
                kacc = psc.tile([128, 2, 512], F32, tag="c", name="k0acc")
                for c in range(KC):
                    for n in range(2):
                        nc.tensor.matmul(
                            qacc[:, n, :], wq[:, c, 0:128], xT[:, c, ts(n, 512)],
                            start=(c == 0), stop=(c == KC - 1),
                        )
                    for n in range(2):
                        nc.tensor.matmul(
                            kacc[:, n, :], wk[:, c, 0:128], xT[:, c, ts(n, 512)],
                            start=(c == 0), stop=(c == KC - 1),
                        )
                nc.vector.tensor_copy(
                    qT[:, 0, 0:1024], qacc[:].rearrange("p a n -> p (a n)")
                )
                nc.vector.tensor_copy(
                    kT[:, 0, 0:1024], kacc[:].rearrange("p a n -> p (a n)")
                )

            def emit_v(jt):
                # v projection for one j-tile: psum [128 tokens, 384]
                pv = psc.tile([128, 2, 512], F32, tag="c", name=f"pv{jt}")
                pvf = pv[:].rearrange("p a n -> p (a n)")[:, 0:HPC * HEAD_DIM]
                for c in range(KC):
                    nc.tensor.matmul(
                        pvf, xT[:, c, ts(jt, 128)], wv[:, c, :],
                        start=(c == 0), stop=(c == KC - 1),
                    )
                nc.vector.tensor_copy(
                    v[:, jt, :, 0:HEAD_DIM],
                    pvf.rearrange("p (h e) -> p h e", h=HPC),
                )

            # ex tiles hold 4 j-tiles each: [128, 4jt, 2head, 2ic, 512]
            def emit_scores_exp(pr_, ic2, jt, ex):
                ss = pss.tile([128, 2, 2, 512], F32, tag="s", name=f"ss{jt}")
                for a_ in range(2):
                    po = 64 * a_
                    for n in range(2):
                        nc.tensor.matmul(
                            ss[:, a_, n, :],
                            kT[po:po + 64, pr_, ts(jt, 128)],
                            qT[po:po + 64, pr_, ts(2 * ic2 + n, 512)],
                            start=True, stop=True,
                        )
                nc.scalar.activation(
                    ex[:, jt % 2, :, :, :], ss[:],
                    mybir.ActivationFunctionType.Exp,
                    scale=1.0 / np.sqrt(HEAD_DIM),
                )
                if use_mask:
                    nc.vector.tensor_scalar_mul(
                        ex[:, jt % 2, :, :, :], ex[:, jt % 2, :, :, :],
                        em[:, jt:jt + 1],
                    )

            def emit_ctx(pr_, pcs, jt, exs):
                ex = exs[jt // 2]
                for a_ in range(2):
                    for n in range(2):
                        nc.tensor.matmul(
                            pcs[a_][0:VW, n, :], v[:, jt, 2 * pr_ + a_, :],
                            ex[:, jt % 2, a_, n, :],
                            start=(jt == 0), stop=(jt == NJ - 1),
                        )

            def emit_finalize(pr_, ic2, pcs):
                # Free the "c" psum ring fast: stage ctx rows (bf16) and the
                # sumexp rows (f32) to SBUF with three quick DVE copies per
                # head (~2.3us), then normalize off-ring: DVE reciprocal of
                # the staged sumexp, multiplies on the idle gpsimd engine,
                # DMA out. All SBUF-SBUF operands start at partition 0 (the
                # walrus verifier requires aligned SB start partitions; psum
                # sources are exempt).
                stages = []
                for a_ in range(2):
                    sl = stpool.tile([32, 2, 512], BF16, tag="sl")
                    sh = stpool.tile([32, 2, 512], BF16, tag="sh")
                    se = stpool.tile([32, 2, 512], F32, tag="se")
                    nc.vector.tensor_copy(sl[:], pcs[a_][0:32, :, :])
                    nc.vector.tensor_copy(sh[:], pcs[a_][32:64, :, :])
                    nc.vector.tensor_copy(se[:], pcs[a_][64:VW, :, :])
                    stages.append((sl, sh, se))
                for a_ in range(2):
                    h = 2 * pr_ + a_
                    sl, sh, se = stages[a_]
                    o = opool.tile([32, 2, 2, 512], F32, tag="o")
                    for icx in range(2):
                        rc = rpool.tile([32, 512], F32, tag="rc")
                        nc.vector.reciprocal(rc[:], se[:, icx, :])
                        nc.gpsimd.tensor_tensor(
                            o[:, icx, 0, :], sl[:, icx, :], rc[:],
                            op=mybir.AluOpType.mult,
                        )
                        nc.gpsimd.tensor_tensor(
                            o[:, icx, 1, :], sh[:, icx, :], rc[:],
                            op=mybir.AluOpType.mult,
                        )
                    nc.sync.dma_start(
                        out_d[h, 0:32, ts(ic2, 1024)], o[:, :, 0, :]
                    )
                    nc.sync.dma_start(
                        out_d[h, 32:64, ts(ic2, 1024)], o[:, :, 1, :]
                    )

            # ---- schedule ----
            # warm the PE p-state during the input DMA window: ~24 tiny
            # matmuls on the already-memset ones region keep the tensor
            # engine continuously busy so the real projections start at
            # full clock instead of the 2x-slow mid p-state
            warm = pss.tile([128, 4, 512], F32, tag="s", name="warm")
            ones32 = v[:, 0, 0, HEAD_DIM:VW]
            for _ in range(28):
                nc.tensor.matmul(warm[0:32, 0, 0:32], ones32, ones32,
                                 start=True, stop=True)

            # startup: pair0 q/k (DMA-gated window), then pair1 q on the
            # "s" ring before the first scores tile
            emit_qk_first()

            # psc-ring fills placed at chunk boundaries (after the previous
            # finalize, before the next ctx accumulators claim the ring);
            # boundary 0/1 has no finalize, so pair1 starts immediately there
            boundary_fills = {
                1: [lambda: emit_qk_psc(1, 0), lambda: emit_qk_psc(1, 1)],
                2: [lambda: emit_qk_psc(2, 0)],                  # q pair2
                3: [lambda: emit_qk_psc(2, 1)],                  # k pair2
            }

            ch0_fills = [
                lambda: emit_qk_psc(0, 0, halves=(1,)),
                lambda: emit_qk_psc(0, 1, halves=(1,)),
            ] + [(lambda j=j: emit_v(j)) for j in range(NJ)]

            prev = None  # (pr, ic2, pcs, exs) of previous chunk
            for CH in range(2 * NPAIR):
                pr_, ic2 = CH // 2, CH % 2
                for fill in boundary_fills.get(CH, []):
                    fill()
                pcs = None
                if prev is not None:
                    pcs = [psc.tile([128, 2, 512], F32, tag="c",
                                    name=f"pc{CH}_{a}") for a in range(2)]
                exs = []
                for jt in range(NJ):
                    if jt % 2 == 0:
                        ex = expool.tile([128, 2, 2, 2, 512], BF16, tag="e")
                        exs.append(ex)
                    emit_scores_exp(pr_, ic2, jt, exs[jt // 2])
                    if CH == 0:
                        ch0_fills[jt]()
                    if prev is not None:
                        emit_ctx(prev[0], pcs, jt, prev[3])
                if CH == 0:
                    for fill in ch0_fills[NJ:]:
                        fill()
                if prev is not None:
                    emit_finalize(prev[0], prev[1], pcs)
                prev = (pr_, ic2, pcs, exs)

            # last chunk's ctx + finalize (trails the act stream closely)
            pcs = [psc.tile([128, 2, 512], F32, tag="c", name=f"pcL_{a}")
                   for a in range(2)]
            for jt in range(NJ):
                emit_ctx(prev[0], pcs, jt, prev[3])
            emit_finalize(prev[0], prev[1], pcs)

    nc.compile()
    return nc


def _enable_ldw_opt():
    # The default backend options carry --enable-ldw-opt=false, which makes
    # walrus emit one LDWEIGHTS per matmul even when consecutive matmuls
    # share the stationary operand. Flip it for this process's compiles.
    from concourse import compiler_utils

    flags = compiler_utils.get_compiler_flags()
    patched = [f.replace("--enable-ldw-opt=false", "--enable-ldw-opt=true")
               for f in flags]
    if patched != flags:
        compiler_utils.set_compiler_flags(patched)


def _get_nc(use_mask: bool, use_bias: bool):
    key = (use_mask, use_bias)
    if key not in _cache:
        if os.environ.get("KERNEL_LDW_OPT"):
            _enable_ldw_opt()
        _cache[key] = _build(use_mask, use_bias)
    return _cache[key]


def kernel(hidden_states, attention_mask, Wq, bq, Wk, bk, Wv, bv):
    global last_results
    hidden_states = np.asarray(hidden_states, dtype=np.float32)
    attention_mask = np.asarray(attention_mask, dtype=np.float32)
    Wq = np.asarray(Wq, dtype=np.float32)
    Wk = np.asarray(Wk, dtype=np.float32)
    Wv = np.asarray(Wv, dtype=np.float32)
    bq = np.asarray(bq, dtype=np.float32)
    bk = np.asarray(bk, dtype=np.float32)
    bv = np.asarray(bv, dtype=np.float32)

    use_mask = bool(np.any(attention_mask))
    use_bias = bool(np.any(bq) or np.any(bk) or np.any(bv))
    nc = _get_nc(use_mask, use_bias)
    KC = 7 if use_bias else 6
    bf16 = ml_dtypes.bfloat16

    in_maps = []
    for c in range(8):
        b = c // 2
        hg = c % 2
        cs = slice(hg * HPC * HEAD_DIM, (hg + 1) * HPC * HEAD_DIM)

        xT = np.zeros((KC * 128, S), dtype=np.float32)
        xT[:HIDDEN] = hidden_states[b].T
        if use_bias:
            xT[HIDDEN] = 1.0

        def wslice(W, bias):
            w = np.zeros((KC * 128, HPC * HEAD_DIM), dtype=np.float32)
            w[:HIDDEN] = W[:, cs]
            if use_bias:
                w[HIDDEN] = bias[cs]
            return w

        m = {
            "xT": xT.astype(bf16),
            "wq": wslice(Wq, bq).astype(bf16),
            "wk": wslice(Wk, bk).astype(bf16),
            "wv": wslice(Wv, bv).astype(bf16),
        }
        if use_mask:
            em = np.exp(attention_mask[b, 0, 0, :]).astype(np.float32)
            m["em"] = np.ascontiguousarray(em.reshape(NJ, 128).T)
        in_maps.append(m)

    res = run_bass_kernel_spmd(
        nc, in_maps, list(range(8)),
        trace=bool(os.environ.get("KERNEL_TRACE")),
    )
    last_results = res

    out = np.empty((B, S, HIDDEN), dtype=np.float32)
    for c in range(8):
        b = c // 2
        hg = c % 2
        r = res.results[c]["out"]  # [6, 64, 2048]
        out[b, :, hg * HPC * HEAD_DIM:(hg + 1) * HPC * HEAD_DIM] = (
            r.transpose(2, 0, 1).reshape(S, HPC * HEAD_DIM)
        )
    return out



# revision 19
# speedup vs baseline: 1.2068x; 1.2068x over previous
"""BERT self-attention (B=4, S=2048, H=768, 12 heads x d=64) on 8 Trainium2
NeuronCores.

Sharding: core c handles batch b = c//2 and head group hg = c%2 (6 heads).
No cross-core communication; the host scatters inputs and gathers the output.

v3 design (vs v2's 356us):
  The scalar engine's softmax exp is the hard floor: exp of all scores
  (25.2M elems/core) at 1 elem/cyc/partition @1.2GHz = 164us, plus a fixed
  ~293ns per ACTIVATE instruction that does NOT pipeline across instructions
  (probed).  PSUM (8 banks) bounds the ACTIVATE group size: scores ring gets
  6 banks (2 groups x 3 banks, double-buffered), ctx accumulators 2 banks.
  So the plan keeps ACT 100% busy on ~1536-elem groups (~201us) and hides
  everything else under it:
  - scores matmuls are K=64; the two heads of a pair use PE row-groups
    0:64 / 64:128 and run CONCURRENTLY (probed 2x; auto tile_position).
    PE total ~405k cyc = ~169us @2.4GHz < ACT.
  - chunk = (head pair, 512-query quarter): 12 chunks x 16 key-tiles.
    Per jt: 2 concurrent scores matmuls -> 2 ring banks; ACT exps each
    3-bank group into fp16 `ex`.
  - ctx is TRANSPOSED: stationary = ex [128 keys, 128 queries], moving =
    v [128 keys, 66] (cols 64:66 = ones), out = [128 queries, 66] psum.
    Queries land on psum PARTITIONS, so sum(exp) is column 64 and the
    softmax divide is a per-partition-scalar op: reciprocal [128,1] +
    tensor_scalar_mul — no cross-partition broadcast, no DRAM roundtrip.
    The 4 query-tile accumulators of a head share one psum bank; only the
    very first matmul per bank uses start=True (start clears has_written
    for the WHOLE bank — probed), everything else start=False.
  - projections q/k/v are injected into the ring as extra slots (the ACT
    stream skips them; DVE casts them to fp16 sbuf).
  - fp16 everywhere (same PE/DVE speed as bf16, 8x finer mantissa).

Per-core layouts (SBUF [128 partitions x free]):
  xT   [128, KC, 2048] fp16   x[b].T by contraction chunk
  wq/wk/wv [128, KC, 384] fp16
  qT/kT [128, 3, 2048] fp16   per head-pair stacked d-dims (even head p0:64,
                              odd head p64:128)
  v    [128, 16, 6, 66] fp16  token-major v; cols 64:66 of each head = ones
  ring psum: 2 x [128, 3, 512] f32 (6 banks); ctx psum: 2 x [128, 4, 66] (2)

Known framework pitfall (verified in BIR): a DMA reader of a tile waits on
only ONE prior writer's semaphore — never give a DMA-read tile multiple
writers.  (The v3 DRAM z-route did, and raced on first execution.)
"""
import os
from collections import deque

import numpy as np

if not os.environ.get("KERNEL_TRACE"):
    os.environ.setdefault("BASS_NEVER_TRACE", "1")

import concourse.bass as bass
import concourse.mybir as mybir
import concourse.tile as tile
from concourse import bacc
from concourse.bass import ts
from concourse.bass_utils import run_bass_kernel_spmd

F32 = mybir.dt.float32
F16 = mybir.dt.float16

HIDDEN = 768
N_HEADS = 12
HEAD_DIM = 64
B = 4
S = 2048
HPC = 6           # heads per core
NPAIR = HPC // 2  # 3 head pairs
NJ = S // 128     # 16 key tiles per chunk
NQT = 4           # query quarters (512 q each)
NCHUNK = NPAIR * NQT  # 12
CTX_LAG = 40      # ctx pop backlog in ring slots (~13 groups)

_cache = {}
last_results = None


def _build(use_mask: bool, use_bias: bool):
    KC = 7 if use_bias else 6
    nc = bacc.Bacc("TRN2", target_bir_lowering=False, debug=False, num_devices=8)

    xT_d = nc.dram_tensor("xT", [KC * 128, S], F16, kind="ExternalInput")
    wq_d = nc.dram_tensor("wq", [KC * 128, HPC * HEAD_DIM], F16, kind="ExternalInput")
    wk_d = nc.dram_tensor("wk", [KC * 128, HPC * HEAD_DIM], F16, kind="ExternalInput")
    wv_d = nc.dram_tensor("wv", [KC * 128, HPC * HEAD_DIM], F16, kind="ExternalInput")
    if use_mask:
        em_d = nc.dram_tensor("em", [128, NJ], F32, kind="ExternalInput")
    out_d = nc.dram_tensor("out", [HPC, S, HEAD_DIM], F32, kind="ExternalOutput")

    with tile.TileContext(nc) as tc:
        with (
            tc.tile_pool(name="const", bufs=1) as cpool_,
            tc.tile_pool(name="big", bufs=1) as big,
            tc.tile_pool(name="ex", bufs=16) as expool,
            tc.tile_pool(name="zr", bufs=8) as zpool,
            tc.tile_pool(name="oo", bufs=6) as opool,
            tc.tile_pool(name="pg", bufs=2, space="PSUM") as gpool,
            tc.tile_pool(name="pc", bufs=2, space="PSUM") as cxpool,
        ):
            if use_mask:
                em = cpool_.tile([128, NJ], F32)
                nc.sync.dma_start(em[:], em_d[:])

            xT = big.tile([128, KC, S], F16)
            wq = big.tile([128, KC, HPC * HEAD_DIM], F16)
            wk = big.tile([128, KC, HPC * HEAD_DIM], F16)
            wv = big.tile([128, KC, HPC * HEAD_DIM], F16)
            qT = big.tile([128, NPAIR, S], F16)
            kT = big.tile([128, NPAIR, S], F16)
            v = big.tile([128, NJ, HPC, 66], F16)
            wsrc = cpool_.tile([32, 512], F16)

            # input DMA: pair-0 k/q weight columns first, then x, then the
            # rest, so the startup projections are gated only by x.
            for c in range(KC):
                nc.sync.dma_start(wk[:, c, 0:128], wk_d[ts(c, 128), 0:128])
                nc.sync.dma_start(wq[:, c, 0:128], wq_d[ts(c, 128), 0:128])
            for c in range(KC):
                nc.sync.dma_start(xT[:, c, :], xT_d[ts(c, 128), :])
            for c in range(KC):
                nc.sync.dma_start(wk[:, c, 128:384], wk_d[ts(c, 128), 128:384])
                nc.sync.dma_start(wq[:, c, 128:384], wq_d[ts(c, 128), 128:384])
            for c in range(KC):
                nc.sync.dma_start(wv[:, c, :], wv_d[ts(c, 128), :])

            nc.vector.memset(wsrc[:], 0.125)
            nc.vector.memset(v[:, :, :, HEAD_DIM:66], 1.0)

            # PE p-state warmup + early ACT table load (the exp table DMA
            # costs ~2.7us; trigger it now).  The warm matmuls also write
            # the full width of BOTH ctx psum banks: a psum bank that has
            # never been written misbehaves on the first accumulation chain
            # of a fresh NEFF execution (first-call-only chunk0/h0
            # corruption observed without this).
            dummy = cpool_.tile([32, 32], F16)
            for wb in range(2):
                warm = cxpool.tile([128, 4, 66], F32, tag="c",
                                   name=f"warm{wb}")
                wf = warm[:].rearrange("p a n -> p (a n)")
                for i in range(8):
                    nc.tensor.matmul(wf, wsrc[:, 0:128], wsrc[:, 0:264],
                                     start=True, stop=True)
                    if wb == 0 and i == 1:
                        nc.scalar.activation(dummy[:], warm[0:32, 0, 0:32],
                                             mybir.ActivationFunctionType.Exp,
                                             scale=0.125)

            # ---- event stream ----
            # ("s", chunk, jt, h) scores slot; ("q"/"k", pair, tt) or
            # ("v", jt) projection slot.  Chunk c = pair*4 + quarter.
            def chunk_events(c):
                ev = [("s", c, jt, h) for jt in range(NJ) for h in (0, 1)]
                inj = []
                if c == 0:
                    inj = [("q", 0, 1)] + [("v", j) for j in range(8)]
                elif c == 1:
                    inj = [("q", 0, 2)] + [("v", j) for j in range(8, NJ)]
                elif c == 2:
                    inj = [("q", 0, 3), ("k", 1, 0), ("k", 1, 1)]
                elif c == 3:
                    inj = [("q", 1, 0), ("k", 1, 2), ("k", 1, 3)]
                elif c == 6:
                    inj = [("q", 1, 3), ("k", 2, 0), ("k", 2, 1)]
                elif c == 7:
                    inj = [("q", 2, 0), ("k", 2, 2), ("k", 2, 3)]
                elif c < 11:
                    inj = [("q", *divmod(c + 1, 4))]
                # v tiles front-packed (ctx of the lagging chunk needs them
                # soon); q/k spread evenly through the chunk
                out = list(ev)
                vs = [e for e in inj if e[0] == "v"]
                rest = [e for e in inj if e[0] != "v"]
                for i, e in enumerate(vs):
                    out.insert(min(1 + 3 * i, len(out)), e)
                n = len(rest)
                for i, e in enumerate(rest):
                    pos = (i + 1) * (len(out) + 1) // (n + 1)
                    out.insert(min(pos, len(out)), e)
                return out

            events = [("k", 0, 0), ("k", 0, 1), ("k", 0, 2), ("k", 0, 3),
                      ("q", 0, 0)]
            for c in range(NCHUNK):
                events.extend(chunk_events(c))

            # nudge proj slots off group-middle positions so ACT runs split
            # as little as possible.  Only ever move a proj EARLIER: moving
            # one later can put it behind a scores event that reads its
            # output, and the tile framework orders by emission — the scores
            # matmul would read uninitialized SBUF (caught by CoreSim as
            # exactly that; on hardware it poisons only the FIRST execution
            # because later runs see the previous run's identical values).
            for i in range(1, len(events)):
                if (i % 3 == 1 and events[i][0] != "s"
                        and events[i - 1][0] == "s"):
                    events[i], events[i - 1] = events[i - 1], events[i]

            def emit_fill(g, pos, ev):
                kind = ev[0]
                if kind == "s":
                    _, c, jt, h = ev
                    p, qtr = divmod(c, 4)
                    po = 64 * h
                    nc.tensor.matmul(
                        g[:, pos, :], kT[po:po + 64, p, ts(jt, 128)],
                        qT[po:po + 64, p, ts(qtr, 512)],
                        start=True, stop=True,
                    )
                elif kind == "v":
                    _, jt = ev
                    for c_ in range(KC):
                        nc.tensor.matmul(
                            g[:, pos, 0:HPC * HEAD_DIM],
                            xT[:, c_, ts(jt, 128)], wv[:, c_, :],
                            start=(c_ == 0), stop=(c_ == KC - 1),
                        )
                    nc.vector.tensor_copy(
                        v[:, jt, :, 0:HEAD_DIM],
                        g[:, pos, 0:HPC * HEAD_DIM].rearrange(
                            "p (h e) -> p h e", h=HPC),
                    )  # cols 64:66 stay the memset ones
                else:
                    _, p, tt = ev
                    w_, dst = (wq, qT) if kind == "q" else (wk, kT)
                    for c_ in range(KC):
                        nc.tensor.matmul(
                            g[:, pos, :], w_[:, c_, ts(p, 128)],
                            xT[:, c_, ts(tt, 512)],
                            start=(c_ == 0), stop=(c_ == KC - 1),
                        )
                    nc.vector.tensor_copy(dst[:, p, ts(tt, 512)], g[:, pos, :])

            pending = deque()   # (ex, pos, c, jt, h)
            cxt = {}            # (c, h) -> psum tile [128, 4, 66]
            remaining = {c: 2 * NJ for c in range(NCHUNK)}

            def finalize(c):
                p, qtr = divmod(c, 4)
                for h in (0, 1):
                    cx = cxt.pop((c, h))
                    o = opool.tile([128, NQT, HEAD_DIM], F32, tag="o",
                                   name=f"o{c}_{h}")
                    for qt in range(NQT):
                        zrec = zpool.tile([128, 1], F32, tag="zrec",
                                          name=f"zrec{c}_{h}{qt}")
                        nc.vector.reciprocal(zrec[:], cx[:, qt, 64:65])
                        nc.vector.tensor_scalar_mul(
                            o[:, qt, :], cx[:, qt, 0:HEAD_DIM], zrec[:, 0:1])
                        nc.sync.dma_start(
                            out_d[2 * p + h,
                                  qtr * 512 + qt * 128:qtr * 512 + (qt + 1) * 128,
                                  :],
                            o[:, qt, :])

            def pop_ctx():
                ex, pos, c, jt, h = pending.popleft()
                p = c // 4
                key = (c, h)
                if key not in cxt:
                    cxt[key] = cxpool.tile([128, NQT, 66], F32, tag="c",
                                           name=f"cx{c}_{h}")
                cx = cxt[key]
                for qt in range(NQT):
                    nc.tensor.matmul(
                        cx[:, qt, :], ex[:, pos, ts(qt, 128)],
                        v[:, jt, 2 * p + h, :],
                        start=(jt == 0 and qt == 0),
                        stop=(jt == NJ - 1 and qt == NQT - 1),
                        skip_group_check=True,
                    )
                remaining[c] -= 1
                if remaining[c] == 0:
                    finalize(c)

            # ---- main ring loop ----
            for base in range(0, len(events), 3):
                grp = events[base:base + 3]
                g = gpool.tile([128, 3, 512], F32, tag="g",
                               name=f"g{base}")
                ex = None
                for pos, ev in enumerate(grp):
                    emit_fill(g, pos, ev)
                # exp the maximal scores runs of this group
                run = None
                runs = []
                for pos, ev in enumerate(grp):
                    if ev[0] == "s":
                        if run is None:
                            run = [pos, pos + 1]
                        else:
                            run[1] = pos + 1
                    else:
                        if run is not None:
                            runs.append(run)
                        run = None
                if run is not None:
                    runs.append(run)
                if runs:
                    ex = expool.tile([128, 3, 512], F16, tag="e",
                                     name=f"ex{base}")
                for a, b_ in runs:
                    nc.scalar.activation(
                        ex[:, a:b_, :], g[:, a:b_, :],
                        mybir.ActivationFunctionType.Exp,
                        scale=1.0 / np.sqrt(HEAD_DIM),
                    )
                    if use_mask:
                        for pos in range(a, b_):
                            _, c, jt, h = grp[pos]
                            nc.vector.tensor_scalar_mul(
                                ex[:, pos, :], ex[:, pos, :],
                                em[:, jt:jt + 1],
                            )
                for pos, ev in enumerate(grp):
                    if ev[0] == "s":
                        pending.append((ex, pos, ev[1], ev[2], ev[3]))
                while len(pending) > CTX_LAG:
                    pop_ctx()

            while pending:
                pop_ctx()

    nc.compile()
    return nc


def _get_nc(use_mask: bool, use_bias: bool):
    key = (use_mask, use_bias)
    if key not in _cache:
        _cache[key] = _build(use_mask, use_bias)
    return _cache[key]


def kernel(hidden_states, attention_mask, Wq, bq, Wk, bk, Wv, bv):
    global last_results
    hidden_states = np.asarray(hidden_states, dtype=np.float32)
    attention_mask = np.asarray(attention_mask, dtype=np.float32)
    Wq = np.asarray(Wq, dtype=np.float32)
    Wk = np.asarray(Wk, dtype=np.float32)
    Wv = np.asarray(Wv, dtype=np.float32)
    bq = np.asarray(bq, dtype=np.float32)
    bk = np.asarray(bk, dtype=np.float32)
    bv = np.asarray(bv, dtype=np.float32)

    use_mask = bool(np.any(attention_mask))
    use_bias = bool(np.any(bq) or np.any(bk) or np.any(bv))
    nc = _get_nc(use_mask, use_bias)
    KC = 7 if use_bias else 6

    in_maps = []
    for c in range(8):
        b = c // 2
        hg = c % 2
        cs = slice(hg * HPC * HEAD_DIM, (hg + 1) * HPC * HEAD_DIM)

        xT = np.zeros((KC * 128, S), dtype=np.float32)
        xT[:HIDDEN] = hidden_states[b].T
        if use_bias:
            xT[HIDDEN] = 1.0

        def wslice(W, bias):
            w = np.zeros((KC * 128, HPC * HEAD_DIM), dtype=np.float32)
            w[:HIDDEN] = W[:, cs]
            if use_bias:
                w[HIDDEN] = bias[cs]
            return w

        m = {
            "xT": xT.astype(np.float16),
            "wq": wslice(Wq, bq).astype(np.float16),
            "wk": wslice(Wk, bk).astype(np.float16),
            "wv": wslice(Wv, bv).astype(np.float16),
        }
        if use_mask:
            em = np.exp(attention_mask[b, 0, 0, :]).astype(np.float32)
            m["em"] = np.ascontiguousarray(em.reshape(NJ, 128).T)
        in_maps.append(m)

    res = run_bass_kernel_spmd(
        nc, in_maps, list(range(8)),
        trace=bool(os.environ.get("KERNEL_TRACE")),
    )
    last_results = res

    out = np.empty((B, S, HIDDEN), dtype=np.float32)
    for c in range(8):
        b = c // 2
        hg = c % 2
        r = res.results[c]["out"]  # [6, 2048, 64]
        out[b, :, hg * HPC * HEAD_DIM:(hg + 1) * HPC * HEAD_DIM] = (
            r.transpose(1, 0, 2).reshape(S, HPC * HEAD_DIM)
        )
    return out


# revision 24
# speedup vs baseline: 1.2438x; 1.0307x over previous
"""BERT self-attention (B=4, S=2048, H=768, 12 heads x d=64) on 8 Trainium2
NeuronCores.

Sharding: core c handles batch b = c//2 and head group hg = c%2 (6 heads).
No cross-core communication; the host scatters inputs and gathers the output.

v3 design (vs v2's 356us):
  The scalar engine's softmax exp is the hard floor: exp of all scores
  (25.2M elems/core) at 1 elem/cyc/partition @1.2GHz = 164us, plus a fixed
  ~293ns per ACTIVATE instruction that does NOT pipeline across instructions
  (probed).  PSUM (8 banks) bounds the ACTIVATE group size: scores ring gets
  6 banks (2 groups x 3 banks, double-buffered), ctx accumulators 2 banks.
  So the plan keeps ACT 100% busy on ~1536-elem groups (~201us) and hides
  everything else under it:
  - scores matmuls are K=64; the two heads of a pair use PE row-groups
    0:64 / 64:128 and run CONCURRENTLY (probed 2x; auto tile_position).
    PE total ~405k cyc = ~169us @2.4GHz < ACT.
  - chunk = (head pair, 512-query quarter): 12 chunks x 16 key-tiles.
    Per jt: 2 concurrent scores matmuls -> 2 ring banks; ACT exps each
    3-bank group into fp16 `ex`.
  - ctx is TRANSPOSED: stationary = ex [128 keys, 128 queries], moving =
    v [128 keys, 66] (cols 64:66 = ones), out = [128 queries, 66] psum.
    Queries land on psum PARTITIONS, so sum(exp) is column 64 and the
    softmax divide is a per-partition-scalar op: reciprocal [128,1] +
    tensor_scalar_mul — no cross-partition broadcast, no DRAM roundtrip.
    The 4 query-tile accumulators of a head share one psum bank; only the
    very first matmul per bank uses start=True (start clears has_written
    for the WHOLE bank — probed), everything else start=False.
  - projections q/k/v are injected into the ring as extra slots (the ACT
    stream skips them; DVE casts them to fp16 sbuf).
  - fp16 everywhere (same PE/DVE speed as bf16, 8x finer mantissa).

Per-core layouts (SBUF [128 partitions x free]):
  xT   [128, KC, 2048] fp16   x[b].T by contraction chunk
  wq/wk/wv [128, KC, 384] fp16
  qT/kT [128, 3, 2048] fp16   per head-pair stacked d-dims (even head p0:64,
                              odd head p64:128)
  v    [128, 16, 6, 66] fp16  token-major v; cols 64:66 of each head = ones
  ring psum: 2 x [128, 3, 512] f32 (6 banks); ctx psum: 2 x [128, 4, 66] (2)

Known framework pitfall (verified in BIR): a DMA reader of a tile waits on
only ONE prior writer's semaphore — never give a DMA-read tile multiple
writers.  (The v3 DRAM z-route did, and raced on first execution.)
"""
import os
from collections import deque

import numpy as np

if not os.environ.get("KERNEL_TRACE"):
    os.environ.setdefault("BASS_NEVER_TRACE", "1")

import concourse.bass as bass
import concourse.mybir as mybir
import concourse.tile as tile
from concourse import bacc
from concourse.bass import ts
from concourse.bass_utils import run_bass_kernel_spmd

F32 = mybir.dt.float32
F16 = mybir.dt.float16

HIDDEN = 768
N_HEADS = 12
HEAD_DIM = 64
B = 4
S = 2048
HPC = 6           # heads per core
NPAIR = HPC // 2  # 3 head pairs
NJ = S // 128     # 16 key tiles per chunk
NQT = 4           # query quarters (512 q each)
NCHUNK = NPAIR * NQT  # 12
CTX_LAG = 28      # ctx pop backlog in ring slots (~9 groups)

_cache = {}
last_results = None


def _build(use_mask: bool, use_bias: bool):
    KC = 7 if use_bias else 6
    nc = bacc.Bacc("TRN2", target_bir_lowering=False, debug=False, num_devices=8)

    xT_d = nc.dram_tensor("xT", [KC * 128, S], F16, kind="ExternalInput")
    wq_d = nc.dram_tensor("wq", [KC * 128, HPC * HEAD_DIM], F16, kind="ExternalInput")
    wk_d = nc.dram_tensor("wk", [KC * 128, HPC * HEAD_DIM], F16, kind="ExternalInput")
    wv_d = nc.dram_tensor("wv", [KC * 128, HPC * HEAD_DIM], F16, kind="ExternalInput")
    if use_mask:
        em_d = nc.dram_tensor("em", [128, NJ], F32, kind="ExternalInput")
    out_d = nc.dram_tensor("out", [HPC, S, HEAD_DIM], F32, kind="ExternalOutput")

    with tile.TileContext(nc) as tc:
        with (
            tc.tile_pool(name="const", bufs=1) as cpool_,
            tc.tile_pool(name="big", bufs=1) as big,
            tc.tile_pool(name="ex", bufs=16) as expool,
            tc.tile_pool(name="zr", bufs=8) as zpool,
            tc.tile_pool(name="oo", bufs=6) as opool,
            tc.tile_pool(name="pg", bufs=2, space="PSUM") as gpool,
            tc.tile_pool(name="pc", bufs=2, space="PSUM") as cxpool,
        ):
            if use_mask:
                em = cpool_.tile([128, NJ], F32)
                nc.sync.dma_start(em[:], em_d[:])

            xT = big.tile([128, KC, S], F16)
            wq = big.tile([128, KC, HPC * HEAD_DIM], F16)
            wk = big.tile([128, KC, HPC * HEAD_DIM], F16)
            wv = big.tile([128, KC, HPC * HEAD_DIM], F16)
            qT = big.tile([128, NPAIR, S], F16)
            kT = big.tile([128, NPAIR, S], F16)
            v = big.tile([128, NJ, HPC, 66], F16)
            wsrc = cpool_.tile([32, 512], F16)

            # warm the PE p-state and trigger the exp ACT-table load
            # (~2.7us) immediately, before anything else queues.
            nc.vector.memset(wsrc[:], 0.125)
            dummy = cpool_.tile([32, 32], F16)
            for wb in range(2):
                warm = cxpool.tile([128, 4, 66], F32, tag="c",
                                   name=f"warm{wb}")
                wf = warm[:].rearrange("p a n -> p (a n)")
                for i in range(8):
                    nc.tensor.matmul(wf, wsrc[:, 0:128], wsrc[:, 0:264],
                                     start=True, stop=True)
                    if wb == 0 and i == 1:
                        nc.scalar.activation(dummy[:], warm[0:32, 0, 0:32],
                                             mybir.ActivationFunctionType.Exp,
                                             scale=0.125)

            # input DMA: pair-0 k/q weight columns, then x in TOKEN-major
            # stripes (each 512-token stripe carries all contraction
            # chunks), so the first k/q projection tiles are gated on only
            # ~1/4 of x.
            for c in range(KC):
                nc.sync.dma_start(wk[:, c, 0:128], wk_d[ts(c, 128), 0:128])
                nc.sync.dma_start(wq[:, c, 0:128], wq_d[ts(c, 128), 0:128])
            for tt in range(4):
                for c in range(KC):
                    nc.sync.dma_start(xT[:, c, ts(tt, 512)],
                                      xT_d[ts(c, 128), ts(tt, 512)])
            for c in range(KC):
                nc.sync.dma_start(wv[:, c, :], wv_d[ts(c, 128), :])
            for c in range(KC):
                nc.sync.dma_start(wk[:, c, 128:384], wk_d[ts(c, 128), 128:384])
                nc.sync.dma_start(wq[:, c, 128:384], wq_d[ts(c, 128), 128:384])

            nc.vector.memset(v[:, :, :, HEAD_DIM:66], 1.0)

            # ---- event stream ----
            # ("s", chunk, jt, h) scores slot; ("q"/"k", pair, tt) or
            # ("v", jt) projection slot.  Chunk c = pair*4 + quarter.
            def chunk_events(c):
                ev = [("s", c, jt, h) for jt in range(NJ) for h in (0, 1)]
                inj = []
                if c == 0:
                    inj = [("k", 0, 1), ("k", 0, 2), ("k", 0, 3),
                           ("q", 0, 1)] + [("v", j) for j in range(8)]
                elif c == 1:
                    inj = [("q", 0, 2)] + [("v", j) for j in range(8, NJ)]
                elif c == 2:
                    inj = [("q", 0, 3), ("k", 1, 0), ("k", 1, 1)]
                elif c == 3:
                    inj = [("q", 1, 0), ("k", 1, 2), ("k", 1, 3)]
                elif c == 6:
                    inj = [("q", 1, 3), ("k", 2, 0), ("k", 2, 1)]
                elif c == 7:
                    inj = [("q", 2, 0), ("k", 2, 2), ("k", 2, 3)]
                elif c < 11:
                    inj = [("q", *divmod(c + 1, 4))]
                # k/v tiles front-packed (upcoming scores / the lagging
                # chunk's ctx need them soon); q spread evenly
                out = list(ev)
                vs = [e for e in inj if e[0] != "q"]
                rest = [e for e in inj if e[0] == "q"]
                for i, e in enumerate(vs):
                    out.insert(min(1 + 3 * i, len(out)), e)
                n = len(rest)
                for i, e in enumerate(rest):
                    pos = (i + 1) * (len(out) + 1) // (n + 1)
                    out.insert(min(pos, len(out)), e)
                return out

            events = [("k", 0, 0), ("q", 0, 0)]
            for c in range(NCHUNK):
                events.extend(chunk_events(c))

            # nudge proj slots off group-middle positions so ACT runs split
            # as little as possible.  Only ever move a proj EARLIER: moving
            # one later can put it behind a scores event that reads its
            # output, and the tile framework orders by emission — the scores
            # matmul would read uninitialized SBUF (caught by CoreSim as
            # exactly that; on hardware it poisons only the FIRST execution
            # because later runs see the previous run's identical values).
            for i in range(1, len(events)):
                if (i % 3 == 1 and events[i][0] != "s"
                        and events[i - 1][0] == "s"):
                    events[i], events[i - 1] = events[i - 1], events[i]

            # Projection chains are queued as single-matmul closures and
            # drained a couple per scores fill, so a 6-matmul contraction
            # chain never runs as one PE burst that starves the ACT ring.
            # Correctness: a chain is force-flushed before emitting any
            # scores fill / ctx matmul that reads its output (the tile
            # framework orders by emission, so a consumer emitted before
            # its producer would read uninitialized SBUF).
            proj_queue = deque()   # (depkey, closure)
            chain_left = {}        # depkey -> closures still queued

            def enqueue_proj(depkey, fns):
                for f in fns:
                    proj_queue.append((depkey, f))
                chain_left[depkey] = chain_left.get(depkey, 0) + len(fns)

            def drain_proj(n=None, need=None):
                while proj_queue:
                    if need is not None and chain_left.get(need, 0) == 0:
                        return
                    if need is None and n is not None and n <= 0:
                        return
                    k2, f = proj_queue.popleft()
                    f()
                    chain_left[k2] -= 1
                    if n is not None:
                        n -= 1

            def emit_fill(g, pos, ev):
                kind = ev[0]
                if kind == "s":
                    _, c, jt, h = ev
                    p, qtr = divmod(c, 4)
                    drain_proj(need=("k", p, jt // 4))
                    drain_proj(need=("q", p, qtr))
                    po = 64 * h
                    nc.tensor.matmul(
                        g[:, pos, :], kT[po:po + 64, p, ts(jt, 128)],
                        qT[po:po + 64, p, ts(qtr, 512)],
                        start=True, stop=True,
                    )
                    drain_proj(n=2)
                elif kind == "v":
                    _, jt = ev

                    def vmm(c_, g=g, pos=pos, jt=jt):
                        nc.tensor.matmul(
                            g[:, pos, 0:HPC * HEAD_DIM],
                            xT[:, c_, ts(jt, 128)], wv[:, c_, :],
                            start=(c_ == 0), stop=(c_ == KC - 1),
                        )

                    def vcast(g=g, pos=pos, jt=jt):
                        nc.vector.tensor_copy(
                            v[:, jt, :, 0:HEAD_DIM],
                            g[:, pos, 0:HPC * HEAD_DIM].rearrange(
                                "p (h e) -> p h e", h=HPC),
                        )  # cols 64:66 stay the memset ones

                    enqueue_proj(("v", jt),
                                 [(lambda c_=c_: vmm(c_)) for c_ in range(KC)]
                                 + [vcast])
                else:
                    _, p, tt = ev
                    w_, dst = (wq, qT) if kind == "q" else (wk, kT)

                    def pmm(c_, g=g, pos=pos, w_=w_, p=p, tt=tt):
                        nc.tensor.matmul(
                            g[:, pos, :], w_[:, c_, ts(p, 128)],
                            xT[:, c_, ts(tt, 512)],
                            start=(c_ == 0), stop=(c_ == KC - 1),
                        )

                    def pcast(g=g, pos=pos, dst=dst, p=p, tt=tt):
                        nc.vector.tensor_copy(dst[:, p, ts(tt, 512)],
                                              g[:, pos, :])

                    enqueue_proj((kind, p, tt),
                                 [(lambda c_=c_: pmm(c_)) for c_ in range(KC)]
                                 + [pcast])

            pending = deque()   # (ex, pos, c, jt, h)
            cxt = {}            # (c, h) -> psum tile [128, 4, 66]
            remaining = {c: 2 * NJ for c in range(NCHUNK)}

            def finalize(c):
                p, qtr = divmod(c, 4)
                for h in (0, 1):
                    cx = cxt.pop((c, h))
                    o = opool.tile([128, NQT, HEAD_DIM], F32, tag="o",
                                   name=f"o{c}_{h}")
                    for qt in range(NQT):
                        zrec = zpool.tile([128, 1], F32, tag="zrec",
                                          name=f"zrec{c}_{h}{qt}")
                        nc.vector.reciprocal(zrec[:], cx[:, qt, 64:65])
                        nc.vector.tensor_scalar_mul(
                            o[:, qt, :], cx[:, qt, 0:HEAD_DIM], zrec[:, 0:1])
                        nc.sync.dma_start(
                            out_d[2 * p + h,
                                  qtr * 512 + qt * 128:qtr * 512 + (qt + 1) * 128,
                                  :],
                            o[:, qt, :])

            def pop_ctx():
                ex, pos, c, jt, h = pending.popleft()
                p = c // 4
                drain_proj(need=("v", jt))
                key = (c, h)
                if key not in cxt:
                    cxt[key] = cxpool.tile([128, NQT, 66], F32, tag="c",
                                           name=f"cx{c}_{h}")
                cx = cxt[key]
                for qt in range(NQT):
                    nc.tensor.matmul(
                        cx[:, qt, :], ex[:, pos, ts(qt, 128)],
                        v[:, jt, 2 * p + h, :],
                        start=(jt == 0 and qt == 0),
                        stop=(jt == NJ - 1 and qt == NQT - 1),
                        skip_group_check=True,
                    )
                remaining[c] -= 1
                if remaining[c] == 0:
                    finalize(c)

            # ---- main ring loop ----
            for base in range(0, len(events), 3):
                grp = events[base:base + 3]
                g = gpool.tile([128, 3, 512], F32, tag="g",
                               name=f"g{base}")
                ex = None
                for pos, ev in enumerate(grp):
                    emit_fill(g, pos, ev)
                # exp the maximal scores runs of this group
                run = None
                runs = []
                for pos, ev in enumerate(grp):
                    if ev[0] == "s":
                        if run is None:
                            run = [pos, pos + 1]
                        else:
                            run[1] = pos + 1
                    else:
                        if run is not None:
                            runs.append(run)
                        run = None
                if run is not None:
                    runs.append(run)
                if runs:
                    ex = expool.tile([128, 3, 512], F16, tag="e",
                                     name=f"ex{base}")
                for a, b_ in runs:
                    nc.scalar.activation(
                        ex[:, a:b_, :], g[:, a:b_, :],
                        mybir.ActivationFunctionType.Exp,
                        scale=1.0 / np.sqrt(HEAD_DIM),
                    )
                    if use_mask:
                        for pos in range(a, b_):
                            _, c, jt, h = grp[pos]
                            nc.vector.tensor_scalar_mul(
                                ex[:, pos, :], ex[:, pos, :],
                                em[:, jt:jt + 1],
                            )
                for pos, ev in enumerate(grp):
                    if ev[0] == "s":
                        pending.append((ex, pos, ev[1], ev[2], ev[3]))
                while len(pending) > CTX_LAG:
                    pop_ctx()

            drain_proj()
            while pending:
                pop_ctx()

    nc.compile()
    return nc


def _get_nc(use_mask: bool, use_bias: bool):
    key = (use_mask, use_bias)
    if key not in _cache:
        _cache[key] = _build(use_mask, use_bias)
    return _cache[key]


def kernel(hidden_states, attention_mask, Wq, bq, Wk, bk, Wv, bv):
    global last_results
    hidden_states = np.asarray(hidden_states, dtype=np.float32)
    attention_mask = np.asarray(attention_mask, dtype=np.float32)
    Wq = np.asarray(Wq, dtype=np.float32)
    Wk = np.asarray(Wk, dtype=np.float32)
    Wv = np.asarray(Wv, dtype=np.float32)
    bq = np.asarray(bq, dtype=np.float32)
    bk = np.asarray(bk, dtype=np.float32)
    bv = np.asarray(bv, dtype=np.float32)

    use_mask = bool(np.any(attention_mask))
    use_bias = bool(np.any(bq) or np.any(bk) or np.any(bv))
    nc = _get_nc(use_mask, use_bias)
    KC = 7 if use_bias else 6

    in_maps = []
    for c in range(8):
        b = c // 2
        hg = c % 2
        cs = slice(hg * HPC * HEAD_DIM, (hg + 1) * HPC * HEAD_DIM)

        xT = np.zeros((KC * 128, S), dtype=np.float32)
        xT[:HIDDEN] = hidden_states[b].T
        if use_bias:
            xT[HIDDEN] = 1.0

        def wslice(W, bias):
            w = np.zeros((KC * 128, HPC * HEAD_DIM), dtype=np.float32)
            w[:HIDDEN] = W[:, cs]
            if use_bias:
                w[HIDDEN] = bias[cs]
            return w

        m = {
            "xT": xT.astype(np.float16),
            "wq": wslice(Wq, bq).astype(np.float16),
            "wk": wslice(Wk, bk).astype(np.float16),
            "wv": wslice(Wv, bv).astype(np.float16),
        }
        if use_mask:
            em = np.exp(attention_mask[b, 0, 0, :]).astype(np.float32)
            m["em"] = np.ascontiguousarray(em.reshape(NJ, 128).T)
        in_maps.append(m)

    res = run_bass_kernel_spmd(
        nc, in_maps, list(range(8)),
        trace=bool(os.environ.get("KERNEL_TRACE")),
    )
    last_results = res

    out = np.empty((B, S, HIDDEN), dtype=np.float32)
    for c in range(8):
        b = c // 2
        hg = c % 2
        r = res.results[c]["out"]  # [6, 2048, 64]
        out[b, :, hg * HPC * HEAD_DIM:(hg + 1) * HPC * HEAD_DIM] = (
            r.transpose(1, 0, 2).reshape(S, HPC * HEAD_DIM)
        )
    return out


# revision 27
# speedup vs baseline: 1.2968x; 1.0426x over previous
"""BERT self-attention (B=4, S=2048, H=768, 12 heads x d=64) on 8 Trainium2
NeuronCores.

Sharding: core c handles batch b = c//2 and head group hg = c%2 (6 heads).
No cross-core communication; the host scatters inputs and gathers the output.

v3 design (vs v2's 356us):
  The scalar engine's softmax exp is the hard floor: exp of all scores
  (25.2M elems/core) at 1 elem/cyc/partition @1.2GHz = 164us, plus a fixed
  ~293ns per ACTIVATE instruction that does NOT pipeline across instructions
  (probed).  PSUM (8 banks) bounds the ACTIVATE group size: scores ring gets
  6 banks (2 groups x 3 banks, double-buffered), ctx accumulators 2 banks.
  So the plan keeps ACT 100% busy on ~1536-elem groups (~201us) and hides
  everything else under it:
  - scores matmuls are K=64; the two heads of a pair use PE row-groups
    0:64 / 64:128 and run CONCURRENTLY (probed 2x; auto tile_position).
    PE total ~405k cyc = ~169us @2.4GHz < ACT.
  - chunk = (head pair, 512-query quarter): 12 chunks x 16 key-tiles.
    Per jt: 2 concurrent scores matmuls -> 2 ring banks; ACT exps each
    3-bank group into fp16 `ex`.
  - ctx is TRANSPOSED: stationary = ex [128 keys, 128 queries], moving =
    v [128 keys, 66] (cols 64:66 = ones), out = [128 queries, 66] psum.
    Queries land on psum PARTITIONS, so sum(exp) is column 64 and the
    softmax divide is a per-partition-scalar op: reciprocal [128,1] +
    tensor_scalar_mul — no cross-partition broadcast, no DRAM roundtrip.
    The 4 query-tile accumulators of a head share one psum bank; only the
    very first matmul per bank uses start=True (start clears has_written
    for the WHOLE bank — probed), everything else start=False.
  - projections q/k/v are injected into the ring as extra slots (the ACT
    stream skips them; DVE casts them to fp16 sbuf).
  - fp16 everywhere (same PE/DVE speed as bf16, 8x finer mantissa).

Per-core layouts (SBUF [128 partitions x free]):
  xT   [128, KC, 2048] fp16   x[b].T by contraction chunk
  wq/wk/wv [128, KC, 384] fp16
  qT/kT [128, 3, 2048] fp16   per head-pair stacked d-dims (even head p0:64,
                              odd head p64:128)
  v    [128, 16, 6, 66] fp16  token-major v; cols 64:66 of each head = ones
  ring psum: 2 x [128, 3, 512] f32 (6 banks); ctx psum: 2 x [128, 4, 66] (2)

Known framework pitfall (verified in BIR): a DMA reader of a tile waits on
only ONE prior writer's semaphore — never give a DMA-read tile multiple
writers.  (The v3 DRAM z-route did, and raced on first execution.)
"""
import os
from collections import deque

import numpy as np

if not os.environ.get("KERNEL_TRACE"):
    os.environ.setdefault("BASS_NEVER_TRACE", "1")

import concourse.bass as bass
import concourse.mybir as mybir
import concourse.tile as tile
from concourse import bacc
from concourse.bass import ts
from concourse.bass_utils import run_bass_kernel_spmd

F32 = mybir.dt.float32
F16 = mybir.dt.float16

HIDDEN = 768
N_HEADS = 12
HEAD_DIM = 64
B = 4
S = 2048
HPC = 6           # heads per core
NPAIR = HPC // 2  # 3 head pairs
NJ = S // 128     # 16 key tiles per chunk
NQT = 4           # query quarters (512 q each)
NCHUNK = NPAIR * NQT  # 12
CTX_LAG = 28      # ctx pop backlog in ring slots (~9 groups)

_cache = {}
last_results = None


def _build(use_mask: bool, use_bias: bool):
    KC = 7 if use_bias else 6
    nc = bacc.Bacc("TRN2", target_bir_lowering=False, debug=False, num_devices=8)

    xT_d = nc.dram_tensor("xT", [KC * 128, S], F16, kind="ExternalInput")
    wq_d = nc.dram_tensor("wq", [KC * 128, HPC * HEAD_DIM], F16, kind="ExternalInput")
    wk_d = nc.dram_tensor("wk", [KC * 128, HPC * HEAD_DIM], F16, kind="ExternalInput")
    wv_d = nc.dram_tensor("wv", [KC * 128, HPC * HEAD_DIM], F16, kind="ExternalInput")
    if use_mask:
        em_d = nc.dram_tensor("em", [128, NJ], F32, kind="ExternalInput")
    out_d = nc.dram_tensor("out", [HPC, S, HEAD_DIM], F32, kind="ExternalOutput")

    with tile.TileContext(nc) as tc:
        with (
            tc.tile_pool(name="const", bufs=1) as cpool_,
            tc.tile_pool(name="big", bufs=1) as big,
            tc.tile_pool(name="ex", bufs=16) as expool,
            tc.tile_pool(name="zr", bufs=8) as zpool,
            tc.tile_pool(name="oo", bufs=6) as opool,
            tc.tile_pool(name="pg", bufs=2, space="PSUM") as gpool,
            tc.tile_pool(name="pc", bufs=2, space="PSUM") as cxpool,
        ):
            if use_mask:
                em = cpool_.tile([128, NJ], F32)
                nc.sync.dma_start(em[:], em_d[:])

            xT = big.tile([128, KC, S], F16)
            wq = big.tile([128, KC, HPC * HEAD_DIM], F16)
            wk = big.tile([128, KC, HPC * HEAD_DIM], F16)
            wv = big.tile([128, KC, HPC * HEAD_DIM], F16)
            qT = big.tile([128, NPAIR, S], F16)
            kT = big.tile([128, NPAIR, S], F16)
            v = big.tile([128, NJ, HPC, 66], F16)
            wsrc = cpool_.tile([32, 512], F16)

            # input DMA.  The HW DGE executes queued DMAs one at a time
            # (~0.6us each observed), so consolidate into few big transfers
            # and split across the two HWDGE engines (SP + ACT queues).
            # x goes in TOKEN-major stripes (each 512-token stripe carries
            # all contraction chunks) so the first k/q projection tiles are
            # gated on only ~1/4 of x.
            wdview = (wk_d, wq_d, wv_d)

            def w3(w, d, lo, hi, eng):
                eng.dma_start(
                    w[:, :, lo:hi],
                    d[:].rearrange("(c p) n -> p c n", p=128)[:, :, lo:hi])

            w3(wk, wk_d, 0, 128, nc.scalar)
            w3(wq, wq_d, 0, 128, nc.scalar)
            for tt in range(4):
                eng = nc.sync if tt % 2 == 0 else nc.scalar
                eng.dma_start(
                    xT[:, :, ts(tt, 512)],
                    xT_d[:].rearrange("(c p) n -> p c n", p=128)[:, :, ts(tt, 512)])
            w3(wv, wv_d, 0, 384, nc.sync)
            w3(wk, wk_d, 128, 384, nc.scalar)
            w3(wq, wq_d, 128, 384, nc.scalar)

            # warm the PE p-state and trigger the exp ACT-table load
            # (~2.7us) immediately, before anything else queues.
            nc.vector.memset(wsrc[:], 0.125)
            dummy = cpool_.tile([32, 32], F16)
            for wb in range(2):
                warm = cxpool.tile([128, 4, 66], F32, tag="c",
                                   name=f"warm{wb}")
                wf = warm[:].rearrange("p a n -> p (a n)")
                for i in range(8):
                    nc.tensor.matmul(wf, wsrc[:, 0:128], wsrc[:, 0:264],
                                     start=True, stop=True)
                    if wb == 0 and i == 1:
                        nc.scalar.activation(dummy[:], warm[0:32, 0, 0:32],
                                             mybir.ActivationFunctionType.Exp,
                                             scale=0.125)

            nc.vector.memset(v[:, :, :, HEAD_DIM:66], 1.0)

            # ---- event stream ----
            # ("s", chunk, jt, h) scores slot; ("q"/"k", pair, tt) or
            # ("v", jt) projection slot.  Chunk c = pair*4 + quarter.
            def chunk_events(c):
                ev = [("s", c, jt, h) for jt in range(NJ) for h in (0, 1)]
                inj = []
                if c == 0:
                    inj = [("k", 0, 1), ("k", 0, 2), ("k", 0, 3),
                           ("q", 0, 1)] + [("v", j) for j in range(8)]
                elif c == 1:
                    inj = [("q", 0, 2)] + [("v", j) for j in range(8, NJ)]
                elif c == 2:
                    inj = [("q", 0, 3), ("k", 1, 0), ("k", 1, 1)]
                elif c == 3:
                    inj = [("k", 1, 2), ("k", 1, 3), ("q", 1, 0)]
                elif c == 5:
                    inj = [("q", 1, 2), ("k", 2, 0)]
                elif c == 6:
                    inj = [("q", 1, 3), ("k", 2, 1), ("k", 2, 2)]
                elif c == 7:
                    inj = [("q", 2, 0), ("k", 2, 3)]
                elif c < 11:
                    inj = [("q", *divmod(c + 1, 4))]
                # k/v tiles front-packed (upcoming scores / the lagging
                # chunk's ctx need them soon); q spread evenly
                out = list(ev)
                vs = [e for e in inj if e[0] != "q"]
                rest = [e for e in inj if e[0] == "q"]
                for i, e in enumerate(vs):
                    out.insert(min(1 + 3 * i, len(out)), e)
                n = len(rest)
                for i, e in enumerate(rest):
                    pos = (i + 1) * (len(out) + 1) // (n + 1)
                    out.insert(min(pos, len(out)), e)
                return out

            events = [("k", 0, 0), ("q", 0, 0)]
            for c in range(NCHUNK):
                events.extend(chunk_events(c))

            # nudge proj slots off group-middle positions so ACT runs split
            # as little as possible.  Only ever move a proj EARLIER: moving
            # one later can put it behind a scores event that reads its
            # output, and the tile framework orders by emission — the scores
            # matmul would read uninitialized SBUF (caught by CoreSim as
            # exactly that; on hardware it poisons only the FIRST execution
            # because later runs see the previous run's identical values).
            for i in range(1, len(events)):
                if (i % 3 == 1 and events[i][0] != "s"
                        and events[i - 1][0] == "s"):
                    events[i], events[i - 1] = events[i - 1], events[i]

            # Projection chains are queued as single-matmul closures and
            # drained a couple per scores fill, so a 6-matmul contraction
            # chain never runs as one PE burst that starves the ACT ring.
            # Correctness: a chain is force-flushed before emitting any
            # scores fill / ctx matmul that reads its output (the tile
            # framework orders by emission, so a consumer emitted before
            # its producer would read uninitialized SBUF).
            proj_queue = deque()   # (depkey, closure)
            chain_left = {}        # depkey -> closures still queued

            def enqueue_proj(depkey, fns):
                for f in fns:
                    proj_queue.append((depkey, f))
                chain_left[depkey] = chain_left.get(depkey, 0) + len(fns)

            def drain_proj(n=None, need=None):
                while proj_queue:
                    if need is not None and chain_left.get(need, 0) == 0:
                        return
                    if need is None and n is not None and n <= 0:
                        return
                    k2, f = proj_queue.popleft()
                    f()
                    chain_left[k2] -= 1
                    if n is not None:
                        n -= 1

            def emit_fill(g, pos, ev):
                kind = ev[0]
                if kind == "s":
                    _, c, jt, h = ev
                    p, qtr = divmod(c, 4)
                    drain_proj(need=("k", p, jt // 4))
                    drain_proj(need=("q", p, qtr))
                    po = 64 * h
                    nc.tensor.matmul(
                        g[:, pos, :], kT[po:po + 64, p, ts(jt, 128)],
                        qT[po:po + 64, p, ts(qtr, 512)],
                        start=True, stop=True,
                    )
                    drain_proj(n=3)
                elif kind == "v":
                    _, jt = ev

                    def vmm(c_, g=g, pos=pos, jt=jt):
                        nc.tensor.matmul(
                            g[:, pos, 0:HPC * HEAD_DIM],
                            xT[:, c_, ts(jt, 128)], wv[:, c_, :],
                            start=(c_ == 0), stop=(c_ == KC - 1),
                        )

                    def vcast(g=g, pos=pos, jt=jt):
                        nc.vector.tensor_copy(
                            v[:, jt, :, 0:HEAD_DIM],
                            g[:, pos, 0:HPC * HEAD_DIM].rearrange(
                                "p (h e) -> p h e", h=HPC),
                        )  # cols 64:66 stay the memset ones

                    enqueue_proj(("v", jt),
                                 [(lambda c_=c_: vmm(c_)) for c_ in range(KC)]
                                 + [vcast])
                else:
                    _, p, tt = ev
                    w_, dst = (wq, qT) if kind == "q" else (wk, kT)

                    def pmm(c_, g=g, pos=pos, w_=w_, p=p, tt=tt):
                        nc.tensor.matmul(
                            g[:, pos, :], w_[:, c_, ts(p, 128)],
                            xT[:, c_, ts(tt, 512)],
                            start=(c_ == 0), stop=(c_ == KC - 1),
                        )

                    def pcast(g=g, pos=pos, dst=dst, p=p, tt=tt):
                        nc.vector.tensor_copy(dst[:, p, ts(tt, 512)],
                                              g[:, pos, :])

                    enqueue_proj((kind, p, tt),
                                 [(lambda c_=c_: pmm(c_)) for c_ in range(KC)]
                                 + [pcast])

            pending = deque()   # (ex, pos, c, jt, h)
            cxt = {}            # (c, h) -> psum tile [128, 4, 66]
            remaining = {c: 2 * NJ for c in range(NCHUNK)}

            def finalize(c):
                p, qtr = divmod(c, 4)
                for h in (0, 1):
                    cx = cxt.pop((c, h))
                    o = opool.tile([128, NQT, HEAD_DIM], F32, tag="o",
                                   name=f"o{c}_{h}")
                    for qt in range(NQT):
                        zrec = zpool.tile([128, 1], F32, tag="zrec",
                                          name=f"zrec{c}_{h}{qt}")
                        nc.vector.reciprocal(zrec[:], cx[:, qt, 64:65])
                        nc.vector.tensor_scalar_mul(
                            o[:, qt, :], cx[:, qt, 0:HEAD_DIM], zrec[:, 0:1])
                        nc.sync.dma_start(
                            out_d[2 * p + h,
                                  qtr * 512 + qt * 128:qtr * 512 + (qt + 1) * 128,
                                  :],
                            o[:, qt, :])

            def pop_ctx():
                ex, pos, c, jt, h = pending.popleft()
                p = c // 4
                drain_proj(need=("v", jt))
                key = (c, h)
                if key not in cxt:
                    cxt[key] = cxpool.tile([128, NQT, 66], F32, tag="c",
                                           name=f"cx{c}_{h}")
                cx = cxt[key]
                for qt in range(NQT):
                    nc.tensor.matmul(
                        cx[:, qt, :], ex[:, pos, ts(qt, 128)],
                        v[:, jt, 2 * p + h, :],
                        start=(jt == 0 and qt == 0),
                        stop=(jt == NJ - 1 and qt == NQT - 1),
                        skip_group_check=True,
                    )
                remaining[c] -= 1
                if remaining[c] == 0:
                    finalize(c)

            # ---- main ring loop ----
            for base in range(0, len(events), 3):
                grp = events[base:base + 3]
                g = gpool.tile([128, 3, 512], F32, tag="g",
                               name=f"g{base}")
                ex = None
                for pos, ev in enumerate(grp):
                    emit_fill(g, pos, ev)
                # exp the maximal scores runs of this group
                run = None
                runs = []
                for pos, ev in enumerate(grp):
                    if ev[0] == "s":
                        if run is None:
                            run = [pos, pos + 1]
                        else:
                            run[1] = pos + 1
                    else:
                        if run is not None:
                            runs.append(run)
                        run = None
                if run is not None:
                    runs.append(run)
                if runs:
                    ex = expool.tile([128, 3, 512], F16, tag="e",
                                     name=f"ex{base}")
                for a, b_ in runs:
                    nc.scalar.activation(
                        ex[:, a:b_, :], g[:, a:b_, :],
                        mybir.ActivationFunctionType.Exp,
                        scale=1.0 / np.sqrt(HEAD_DIM),
                    )
                    if use_mask:
                        for pos in range(a, b_):
                            _, c, jt, h = grp[pos]
                            nc.vector.tensor_scalar_mul(
                                ex[:, pos, :], ex[:, pos, :],
                                em[:, jt:jt + 1],
                            )
                for pos, ev in enumerate(grp):
                    if ev[0] == "s":
                        pending.append((ex, pos, ev[1], ev[2], ev[3]))
                npop = 0
                while len(pending) > CTX_LAG and npop < 4:
                    pop_ctx()
                    npop += 1

            drain_proj()
            while pending:
                pop_ctx()

    nc.compile()
    return nc


def _get_nc(use_mask: bool, use_bias: bool):
    key = (use_mask, use_bias)
    if key not in _cache:
        _cache[key] = _build(use_mask, use_bias)
    return _cache[key]


def kernel(hidden_states, attention_mask, Wq, bq, Wk, bk, Wv, bv):
    global last_results
    hidden_states = np.asarray(hidden_states, dtype=np.float32)
    attention_mask = np.asarray(attention_mask, dtype=np.float32)
    Wq = np.asarray(Wq, dtype=np.float32)
    Wk = np.asarray(Wk, dtype=np.float32)
    Wv = np.asarray(Wv, dtype=np.float32)
    bq = np.asarray(bq, dtype=np.float32)
    bk = np.asarray(bk, dtype=np.float32)
    bv = np.asarray(bv, dtype=np.float32)

    use_mask = bool(np.any(attention_mask))
    use_bias = bool(np.any(bq) or np.any(bk) or np.any(bv))
    nc = _get_nc(use_mask, use_bias)
    KC = 7 if use_bias else 6

    in_maps = []
    for c in range(8):
        b = c // 2
        hg = c % 2
        cs = slice(hg * HPC * HEAD_DIM, (hg + 1) * HPC * HEAD_DIM)

        xT = np.zeros((KC * 128, S), dtype=np.float32)
        xT[:HIDDEN] = hidden_states[b].T
        if use_bias:
            xT[HIDDEN] = 1.0

        def wslice(W, bias):
            w = np.zeros((KC * 128, HPC * HEAD_DIM), dtype=np.float32)
            w[:HIDDEN] = W[:, cs]
            if use_bias:
                w[HIDDEN] = bias[cs]
            return w

        m = {
            "xT": xT.astype(np.float16),
            "wq": wslice(Wq, bq).astype(np.float16),
            "wk": wslice(Wk, bk).astype(np.float16),
            "wv": wslice(Wv, bv).astype(np.float16),
        }
        if use_mask:
            em = np.exp(attention_mask[b, 0, 0, :]).astype(np.float32)
            m["em"] = np.ascontiguousarray(em.reshape(NJ, 128).T)
        in_maps.append(m)

    res = run_bass_kernel_spmd(
        nc, in_maps, list(range(8)),
        trace=bool(os.environ.get("KERNEL_TRACE")),
    )
    last_results = res

    out = np.empty((B, S, HIDDEN), dtype=np.float32)
    for c in range(8):
        b = c // 2
        hg = c % 2
        r = res.results[c]["out"]  # [6, 2048, 64]
        out[b, :, hg * HPC * HEAD_DIM:(hg + 1) * HPC * HEAD_DIM] = (
            r.transpose(1, 0, 2).reshape(S, HPC * HEAD_DIM)
        )
    return out
